# revision 1
# baseline (speedup 1.0000x reference)
"""Trainium2 Bass kernel for the CP-PINN tensor reconstruction problem.

Computes, for xs (3,320,1) and three per-axis MLP weight stacks:
    f_d = MLP_d(xs[d])            (320, 64)   [tanh MLP: 1->128->128->128->64]
    out[a,b,c] = sum_r f_0[a,r] * f_1[b,r] * f_2[c,r]   ->  (320, 320, 320) f32

Strategy: data-parallel over the output's first axis across 8 NeuronCores
(40 a-points per core, no collectives). Each core:
  - computes the three MLPs in transposed (rank-major) layout on the
    TensorEngine + ScalarEngine (tanh),
  - builds the Khatri-Rao product kr[r, a*320+b] = f0[r,a]*f1[r,b] with
    VectorEngine tensor_scalar ops,
  - reconstructs its (40*320, 320) output slab with 100 matmuls
    kr_chunk(64,128)^T @ f2(64,320), evacuating PSUM via alternating
    Vector/Scalar copies into SBUF staging tiles, and streams them to HBM
    with batched 1.6 MB DMAs (the kernel is HBM-write-bound: 16.4 MB/core).
"""

import sys

if "/opt/trn_rl_repo" not in sys.path:
    sys.path.insert(0, "/opt/trn_rl_repo")

import numpy as np

import concourse.bacc as bacc
import concourse.mybir as mybir
from concourse import tile
from concourse.bass_utils import run_bass_kernel_spmd

DIMS = 3
N = 320          # points per coordinate axis
R = 64           # CP rank
H = 128          # hidden width
NCORES = 8
NA = N // NCORES          # a-points per core (40)
NROWS = NA * N            # output rows per core (12800)
MCH = 128                 # (a,b)-rows per matmul chunk
NCHUNK = NROWS // MCH     # 100
DMA_BATCH = 10            # chunks per output DMA (1.6 MB each)

F32 = mybir.dt.float32
TANH = mybir.ActivationFunctionType.Tanh
IDENT = mybir.ActivationFunctionType.Identity

_PROG = None


def _build_program():
    nc = bacc.Bacc("TRN2", target_bir_lowering=False)

    x0 = nc.dram_tensor("x0", [1, NA], F32, kind="ExternalInput")
    x1 = nc.dram_tensor("x1", [1, N], F32, kind="ExternalInput")
    x2 = nc.dram_tensor("x2", [1, N], F32, kind="ExternalInput")
    w0 = nc.dram_tensor("w0", [DIMS, 1, H], F32, kind="ExternalInput")
    w1 = nc.dram_tensor("w1", [DIMS, H, H], F32, kind="ExternalInput")
    w2 = nc.dram_tensor("w2", [DIMS, H, H], F32, kind="ExternalInput")
    w3 = nc.dram_tensor("w3", [DIMS, H, R], F32, kind="ExternalInput")
    b0 = nc.dram_tensor("b0", [DIMS, H, 1], F32, kind="ExternalInput")
    b1 = nc.dram_tensor("b1", [DIMS, H, 1], F32, kind="ExternalInput")
    b2 = nc.dram_tensor("b2", [DIMS, H, 1], F32, kind="ExternalInput")
    b3 = nc.dram_tensor("b3", [DIMS, R, 1], F32, kind="ExternalInput")
    out = nc.dram_tensor("out", [NROWS, N], F32, kind="ExternalOutput")

    with tile.TileContext(nc) as tc:
        with (
            tc.tile_pool(name="consts", bufs=1) as consts,
            tc.tile_pool(name="work", bufs=2) as work,
            tc.tile_pool(name="stage", bufs=3) as stagep,
            tc.tile_pool(name="mlp_ps", bufs=2, space="PSUM") as mlp_ps,
            tc.tile_pool(name="cp_ps", bufs=4, space="PSUM") as cp_ps,
        ):
            # Weights/biases for the 3 per-axis MLPs, packed along the free dim.
            w0_sb = consts.tile([1, DIMS * H], F32)
            w1_sb = consts.tile([H, DIMS * H], F32)
            w2_sb = consts.tile([H, DIMS * H], F32)
            w3_sb = consts.tile([H, DIMS * R], F32)
            b0_sb = consts.tile([H, DIMS], F32)
            b1_sb = consts.tile([H, DIMS], F32)
            b2_sb = consts.tile([H, DIMS], F32)
            b3_sb = consts.tile([R, DIMS], F32)
            for d in range(DIMS):
                nc.sync.dma_start(w0_sb[:, d * H:(d + 1) * H], w0[d, :, :])
                nc.sync.dma_start(w1_sb[:, d * H:(d + 1) * H], w1[d, :, :])
                nc.sync.dma_start(w2_sb[:, d * H:(d + 1) * H], w2[d, :, :])
                nc.sync.dma_start(w3_sb[:, d * R:(d + 1) * R], w3[d, :, :])
                nc.sync.dma_start(b0_sb[:, d:d + 1], b0[d, :, :])
                nc.sync.dma_start(b1_sb[:, d:d + 1], b1[d, :, :])
                nc.sync.dma_start(b2_sb[:, d:d + 1], b2[d, :, :])
                nc.sync.dma_start(b3_sb[:, d:d + 1], b3[d, :, :])

            f0_sb = consts.tile([R, NA], F32)
            f1_sb = consts.tile([R, N], F32)
            f2_sb = consts.tile([R, N], F32)

            def mlp(d, x_dram, npts, f_sb):
                # All intermediates are kept transposed: (features, points).
                xt = work.tile([1, npts], F32, name=f"xt_{d}", tag="xt")
                nc.sync.dma_start(xt[:], x_dram[:, :])
                ps = mlp_ps.tile([H, npts], F32, name=f"ps0_{d}", tag="mlp_ps")
                nc.tensor.matmul(ps[:], w0_sb[:, d * H:(d + 1) * H], xt[:],
                                 start=True, stop=True)
                h = work.tile([H, npts], F32, name=f"h0_{d}", tag="h")
                nc.scalar.activation(h[:], ps[:], TANH, bias=b0_sb[:, d:d + 1])
                for li, (w_sb, b_sb) in enumerate(((w1_sb, b1_sb), (w2_sb, b2_sb))):
                    ps = mlp_ps.tile([H, npts], F32, name=f"ps{li + 1}_{d}", tag="mlp_ps")
                    nc.tensor.matmul(ps[:], w_sb[:, d * H:(d + 1) * H], h[:],
                                     start=True, stop=True)
                    h = work.tile([H, npts], F32, name=f"h{li + 1}_{d}", tag="h")
                    nc.scalar.activation(h[:], ps[:], TANH, bias=b_sb[:, d:d + 1])
                ps = mlp_ps.tile([R, npts], F32, name=f"psf_{d}", tag="mlp_ps")
                nc.tensor.matmul(ps[:], w3_sb[:, d * R:(d + 1) * R], h[:],
                                 start=True, stop=True)
                nc.scalar.activation(f_sb[:], ps[:], IDENT, bias=b3_sb[:, d:d + 1])

            mlp(0, x0, NA, f0_sb)
            mlp(1, x1, N, f1_sb)
            mlp(2, x2, N, f2_sb)

            # Khatri-Rao: kr[r, a*N + b] = f0[r, a] * f1[r, b]
            kr_sb = consts.tile([R, NROWS], F32)
            for a in range(NA):
                nc.vector.tensor_scalar_mul(
                    kr_sb[:, a * N:(a + 1) * N], f1_sb[:], f0_sb[:, a:a + 1])

            # CP reconstruction: out rows in chunks of 128, batched DMAs out.
            outv = out[:, :].rearrange("(m p) c -> p m c", p=MCH)
            for g in range(NCHUNK // DMA_BATCH):
                stg = stagep.tile([MCH, DMA_BATCH * N], F32, name="stg", tag="stg")
                for k in range(DMA_BATCH):
                    m = g * DMA_BATCH + k
                    ps = cp_ps.tile([MCH, N], F32, name="cps", tag="cps")
                    nc.tensor.matmul(ps[:], kr_sb[:, m * MCH:(m + 1) * MCH],
                                     f2_sb[:], start=True, stop=True)
                    if k % 2 == 0:
                        nc.vector.tensor_copy(stg[:, k * N:(k + 1) * N], ps[:])
                    else:
                        nc.scalar.copy(stg[:, k * N:(k + 1) * N], ps[:])
                nc.sync.dma_start(
                    outv[:, g * DMA_BATCH:(g + 1) * DMA_BATCH, :],
                    stg.rearrange("p (m c) -> p m c", c=N),
                )

    nc.compile()
    return nc


def _get_program():
    global _PROG
    if _PROG is None:
        _PROG = _build_program()
    return _PROG


def _make_in_maps(xs, W0, b0, W1, b1, W2, b2, W3, b3):
    f = lambda x: np.ascontiguousarray(np.asarray(x), dtype=np.float32)
    xs = f(xs)
    base = {
        "x1": f(xs[1, :, 0].reshape(1, N)),
        "x2": f(xs[2, :, 0].reshape(1, N)),
        "w0": f(W0), "w1": f(W1), "w2": f(W2), "w3": f(W3),
        "b0": f(np.asarray(b0).reshape(DIMS, H, 1)),
        "b1": f(np.asarray(b1).reshape(DIMS, H, 1)),
        "b2": f(np.asarray(b2).reshape(DIMS, H, 1)),
        "b3": f(np.asarray(b3).reshape(DIMS, R, 1)),
    }
    in_maps = []
    for i in range(NCORES):
        m = dict(base)
        m["x0"] = f(xs[0, i * NA:(i + 1) * NA, 0].reshape(1, NA))
        in_maps.append(m)
    return in_maps


def run_spmd(inputs_kwargs, **run_kwargs):
    """Build (cached) program, run on all 8 cores; returns BassKernelResults."""
    nc = _get_program()
    in_maps = _make_in_maps(**inputs_kwargs)
    return run_bass_kernel_spmd(nc, in_maps, core_ids=list(range(NCORES)),
                                **run_kwargs)


def kernel(xs, W0, b0, W1, b1, W2, b2, W3, b3):
    res = run_spmd(dict(xs=xs, W0=W0, b0=b0, W1=W1, b1=b1,
                        W2=W2, b2=b2, W3=W3, b3=b3))
    slabs = [r["out"].reshape(NA, N, N) for r in res.results]
    return np.concatenate(slabs, axis=0)


# revision 2
# speedup vs baseline: 10.4033x; 10.4033x over previous
"""Trainium2 Bass kernel for the CP-PINN tensor reconstruction problem.

Computes, for xs (3,320,1) and three per-axis MLP weight stacks:
    f_d = MLP_d(xs[d])            (320, 64)   [tanh MLP: 1->128->128->128->64]
    out[a,b,c] = sum_r f_0[a,r] * f_1[b,r] * f_2[c,r]   ->  (320, 320, 320) f32

Strategy: data-parallel over the output's first axis across 8 NeuronCores
(40 a-points per core, no collectives). Each core:
  - computes the three MLPs in transposed (rank-major) layout on the
    TensorEngine + ScalarEngine (tanh),
  - builds the Khatri-Rao product kr[r, a*320+b] = f0[r,a]*f1[r,b] with
    VectorEngine tensor_scalar ops,
  - reconstructs its (40*320, 320) output slab with 100 matmuls
    kr_chunk(64,128)^T @ f2(64,320), evacuating PSUM via alternating
    Vector/Scalar copies into SBUF staging tiles, and streams them to HBM
    with batched 1.6 MB DMAs (the kernel is HBM-write-bound: 16.4 MB/core).
"""

import sys

if "/opt/trn_rl_repo" not in sys.path:
    sys.path.insert(0, "/opt/trn_rl_repo")

import numpy as np

import concourse.bacc as bacc
import concourse.mybir as mybir
from concourse import tile
from concourse.bass_utils import run_bass_kernel_spmd

DIMS = 3
N = 320          # points per coordinate axis
R = 64           # CP rank
H = 128          # hidden width
NCORES = 8
NA = N // NCORES          # a-points per core (40)
NROWS = NA * N            # output rows per core (12800)
MCH = 128                 # (a,b)-rows per matmul chunk
NCHUNK = NROWS // MCH     # 100
DMA_BATCH = 10            # chunks per output DMA (1.6 MB each)

F32 = mybir.dt.float32
TANH = mybir.ActivationFunctionType.Tanh
IDENT = mybir.ActivationFunctionType.Identity

_PROG = None


def _build_program(loop=1):
    """loop>1 wraps the whole compute body in a Tile hardware For_i that
    repeats it `loop` times inside one NEFF launch — benchmarking only."""
    nc = bacc.Bacc("TRN2", target_bir_lowering=False)

    x0 = nc.dram_tensor("x0", [1, NA], F32, kind="ExternalInput")
    x1 = nc.dram_tensor("x1", [1, N], F32, kind="ExternalInput")
    x2 = nc.dram_tensor("x2", [1, N], F32, kind="ExternalInput")
    w0 = nc.dram_tensor("w0", [DIMS, 1, H], F32, kind="ExternalInput")
    w1 = nc.dram_tensor("w1", [DIMS, H, H], F32, kind="ExternalInput")
    w2 = nc.dram_tensor("w2", [DIMS, H, H], F32, kind="ExternalInput")
    w3 = nc.dram_tensor("w3", [DIMS, H, R], F32, kind="ExternalInput")
    b0 = nc.dram_tensor("b0", [DIMS, H, 1], F32, kind="ExternalInput")
    b1 = nc.dram_tensor("b1", [DIMS, H, 1], F32, kind="ExternalInput")
    b2 = nc.dram_tensor("b2", [DIMS, H, 1], F32, kind="ExternalInput")
    b3 = nc.dram_tensor("b3", [DIMS, R, 1], F32, kind="ExternalInput")
    out = nc.dram_tensor("out", [NROWS, N], F32, kind="ExternalOutput")

    with tile.TileContext(nc) as tc:
        with (
            tc.tile_pool(name="consts", bufs=1) as consts,
            tc.tile_pool(name="work", bufs=2) as work,
            tc.tile_pool(name="stage", bufs=3) as stagep,
            tc.tile_pool(name="mlp_ps", bufs=2, space="PSUM") as mlp_ps,
            tc.tile_pool(name="cp_ps", bufs=4, space="PSUM") as cp_ps,
        ):
            # Weights/biases for the 3 per-axis MLPs, packed along the free dim.
            w0_sb = consts.tile([1, DIMS * H], F32)
            w1_sb = consts.tile([H, DIMS * H], F32)
            w2_sb = consts.tile([H, DIMS * H], F32)
            w3_sb = consts.tile([H, DIMS * R], F32)
            b0_sb = consts.tile([H, DIMS], F32)
            b1_sb = consts.tile([H, DIMS], F32)
            b2_sb = consts.tile([H, DIMS], F32)
            b3_sb = consts.tile([R, DIMS], F32)
            for d in range(DIMS):
                nc.sync.dma_start(w0_sb[:, d * H:(d + 1) * H], w0[d, :, :])
                nc.sync.dma_start(w1_sb[:, d * H:(d + 1) * H], w1[d, :, :])
                nc.sync.dma_start(w2_sb[:, d * H:(d + 1) * H], w2[d, :, :])
                nc.sync.dma_start(w3_sb[:, d * R:(d + 1) * R], w3[d, :, :])
                nc.sync.dma_start(b0_sb[:, d:d + 1], b0[d, :, :])
                nc.sync.dma_start(b1_sb[:, d:d + 1], b1[d, :, :])
                nc.sync.dma_start(b2_sb[:, d:d + 1], b2[d, :, :])
                nc.sync.dma_start(b3_sb[:, d:d + 1], b3[d, :, :])

            import contextlib
            loop_cm = (tc.For_i(0, loop, 1,
                                hint_engines=(mybir.EngineType.PE,))
                       if loop > 1 else contextlib.nullcontext())
            with loop_cm:
                _emit_body(nc, tc, consts, work, stagep, mlp_ps, cp_ps,
                           x0, x1, x2, out,
                           w0_sb, w1_sb, w2_sb, w3_sb,
                           b0_sb, b1_sb, b2_sb, b3_sb)

    nc.compile()
    return nc


def _emit_body(nc, tc, consts, work, stagep, mlp_ps, cp_ps,
               x0, x1, x2, out,
               w0_sb, w1_sb, w2_sb, w3_sb, b0_sb, b1_sb, b2_sb, b3_sb):
    if True:
        if True:
            f0_sb = consts.tile([R, NA], F32)
            f1_sb = consts.tile([R, N], F32)
            f2_sb = consts.tile([R, N], F32)

            def mlp(d, x_dram, npts, f_sb):
                # All intermediates are kept transposed: (features, points).
                xt = work.tile([1, npts], F32, name=f"xt_{d}", tag="xt")
                nc.sync.dma_start(xt[:], x_dram[:, :])
                ps = mlp_ps.tile([H, npts], F32, name=f"ps0_{d}", tag="mlp_ps")
                nc.tensor.matmul(ps[:], w0_sb[:, d * H:(d + 1) * H], xt[:],
                                 start=True, stop=True)
                h = work.tile([H, npts], F32, name=f"h0_{d}", tag="h")
                nc.scalar.activation(h[:], ps[:], TANH, bias=b0_sb[:, d:d + 1])
                for li, (w_sb, b_sb) in enumerate(((w1_sb, b1_sb), (w2_sb, b2_sb))):
                    ps = mlp_ps.tile([H, npts], F32, name=f"ps{li + 1}_{d}", tag="mlp_ps")
                    nc.tensor.matmul(ps[:], w_sb[:, d * H:(d + 1) * H], h[:],
                                     start=True, stop=True)
                    h = work.tile([H, npts], F32, name=f"h{li + 1}_{d}", tag="h")
                    nc.scalar.activation(h[:], ps[:], TANH, bias=b_sb[:, d:d + 1])
                ps = mlp_ps.tile([R, npts], F32, name=f"psf_{d}", tag="mlp_ps")
                nc.tensor.matmul(ps[:], w3_sb[:, d * R:(d + 1) * R], h[:],
                                 start=True, stop=True)
                nc.scalar.activation(f_sb[:], ps[:], IDENT, bias=b3_sb[:, d:d + 1])

            mlp(0, x0, NA, f0_sb)
            mlp(1, x1, N, f1_sb)
            mlp(2, x2, N, f2_sb)

            # Khatri-Rao: kr[r, a*N + b] = f0[r, a] * f1[r, b]
            kr_sb = consts.tile([R, NROWS], F32)
            for a in range(NA):
                nc.vector.tensor_scalar_mul(
                    kr_sb[:, a * N:(a + 1) * N], f1_sb[:], f0_sb[:, a:a + 1])

            # CP reconstruction: out rows in chunks of 128, batched DMAs out.
            outv = out[:, :].rearrange("(m p) c -> p m c", p=MCH)
            for g in range(NCHUNK // DMA_BATCH):
                stg = stagep.tile([MCH, DMA_BATCH * N], F32, name="stg", tag="stg")
                for k in range(DMA_BATCH):
                    m = g * DMA_BATCH + k
                    ps = cp_ps.tile([MCH, N], F32, name="cps", tag="cps")
                    nc.tensor.matmul(ps[:], kr_sb[:, m * MCH:(m + 1) * MCH],
                                     f2_sb[:], start=True, stop=True)
                    if k % 2 == 0:
                        nc.vector.tensor_copy(stg[:, k * N:(k + 1) * N], ps[:])
                    else:
                        nc.scalar.copy(stg[:, k * N:(k + 1) * N], ps[:])
                nc.sync.dma_start(
                    outv[:, g * DMA_BATCH:(g + 1) * DMA_BATCH, :],
                    stg.rearrange("p (m c) -> p m c", c=N),
                )


def _get_program():
    global _PROG
    if _PROG is None:
        _PROG = _build_program()
    return _PROG


def _make_in_maps(xs, W0, b0, W1, b1, W2, b2, W3, b3):
    f = lambda x: np.ascontiguousarray(np.asarray(x), dtype=np.float32)
    xs = f(xs)
    base = {
        "x1": f(xs[1, :, 0].reshape(1, N)),
        "x2": f(xs[2, :, 0].reshape(1, N)),
        "w0": f(W0), "w1": f(W1), "w2": f(W2), "w3": f(W3),
        "b0": f(np.asarray(b0).reshape(DIMS, H, 1)),
        "b1": f(np.asarray(b1).reshape(DIMS, H, 1)),
        "b2": f(np.asarray(b2).reshape(DIMS, H, 1)),
        "b3": f(np.asarray(b3).reshape(DIMS, R, 1)),
    }
    in_maps = []
    for i in range(NCORES):
        m = dict(base)
        m["x0"] = f(xs[0, i * NA:(i + 1) * NA, 0].reshape(1, NA))
        in_maps.append(m)
    return in_maps


def run_spmd(inputs_kwargs, **run_kwargs):
    """Build (cached) program, run on all 8 cores; returns BassKernelResults."""
    nc = _get_program()
    in_maps = _make_in_maps(**inputs_kwargs)
    return run_bass_kernel_spmd(nc, in_maps, core_ids=list(range(NCORES)),
                                **run_kwargs)


def kernel(xs, W0, b0, W1, b1, W2, b2, W3, b3):
    res = run_spmd(dict(xs=xs, W0=W0, b0=b0, W1=W1, b1=b1,
                        W2=W2, b2=b2, W3=W3, b3=b3))
    slabs = [r["out"].reshape(NA, N, N) for r in res.results]
    return np.concatenate(slabs, axis=0)


# revision 3
# speedup vs baseline: 11.6820x; 1.1229x over previous
"""Trainium2 Bass kernel for the CP-PINN tensor reconstruction problem.

Computes, for xs (3,320,1) and three per-axis MLP weight stacks:
    f_d = MLP_d(xs[d])            (320, 64)   [tanh MLP: 1->128->128->128->64]
    out[a,b,c] = sum_r f_0[a,r] * f_1[b,r] * f_2[c,r]   ->  (320, 320, 320) f32

Strategy: data-parallel over the output's first axis across 8 NeuronCores
(40 a-points per core, no collectives). Each core:
  - computes the three MLPs in transposed (rank-major) layout on the
    TensorEngine + ScalarEngine (tanh),
  - builds the Khatri-Rao product kr[r, a*320+b] = f0[r,a]*f1[r,b] with
    VectorEngine tensor_scalar ops,
  - reconstructs its (40*320, 320) output slab with 100 matmuls
    kr_chunk(64,128)^T @ f2(64,320), evacuating PSUM via alternating
    Vector/Scalar copies into SBUF staging tiles, and streams them to HBM
    with batched 1.6 MB DMAs (the kernel is HBM-write-bound: 16.4 MB/core).
"""

import sys

if "/opt/trn_rl_repo" not in sys.path:
    sys.path.insert(0, "/opt/trn_rl_repo")

import numpy as np

import concourse.bacc as bacc
import concourse.mybir as mybir
from concourse import tile
from concourse.bass_utils import run_bass_kernel_spmd

DIMS = 3
N = 320          # points per coordinate axis
R = 64           # CP rank
H = 128          # hidden width
NCORES = 8
NA = N // NCORES          # a-points per core (40)
NROWS = NA * N            # output rows per core (12800)
MCH = 128                 # (a,b)-rows per matmul chunk
NCHUNK = NROWS // MCH     # 100
DMA_BATCH = 10            # chunks per output DMA (1.6 MB each)

F32 = mybir.dt.float32
TANH = mybir.ActivationFunctionType.Tanh
IDENT = mybir.ActivationFunctionType.Identity

_PROG = None


def _build_program(loop=1, variant="full"):
    """loop>1 wraps the whole compute body in a Tile hardware For_i that
    repeats it `loop` times inside one NEFF launch — benchmarking only."""
    nc = bacc.Bacc("TRN2", target_bir_lowering=False)

    x0 = nc.dram_tensor("x0", [1, NA], F32, kind="ExternalInput")
    x1 = nc.dram_tensor("x1", [1, N], F32, kind="ExternalInput")
    x2 = nc.dram_tensor("x2", [1, N], F32, kind="ExternalInput")
    w0 = nc.dram_tensor("w0", [DIMS, 1, H], F32, kind="ExternalInput")
    w1 = nc.dram_tensor("w1", [DIMS, H, H], F32, kind="ExternalInput")
    w2 = nc.dram_tensor("w2", [DIMS, H, H], F32, kind="ExternalInput")
    w3 = nc.dram_tensor("w3", [DIMS, H, R], F32, kind="ExternalInput")
    b0 = nc.dram_tensor("b0", [DIMS, H, 1], F32, kind="ExternalInput")
    b1 = nc.dram_tensor("b1", [DIMS, H, 1], F32, kind="ExternalInput")
    b2 = nc.dram_tensor("b2", [DIMS, H, 1], F32, kind="ExternalInput")
    b3 = nc.dram_tensor("b3", [DIMS, R, 1], F32, kind="ExternalInput")
    out = nc.dram_tensor("out", [NROWS, N], F32, kind="ExternalOutput")

    with tile.TileContext(nc) as tc:
        with (
            tc.tile_pool(name="consts", bufs=1) as consts,
            tc.tile_pool(name="work", bufs=2) as work,
            tc.tile_pool(name="stage", bufs=3) as stagep,
            tc.tile_pool(name="mlp_ps", bufs=2, space="PSUM") as mlp_ps,
            tc.tile_pool(name="cp_ps", bufs=4, space="PSUM") as cp_ps,
        ):
            # Weights/biases for the 3 per-axis MLPs, packed along the free dim.
            w0_sb = consts.tile([1, DIMS * H], F32)
            w1_sb = consts.tile([H, DIMS * H], F32)
            w2_sb = consts.tile([H, DIMS * H], F32)
            w3_sb = consts.tile([H, DIMS * R], F32)
            b0_sb = consts.tile([H, DIMS], F32)
            b1_sb = consts.tile([H, DIMS], F32)
            b2_sb = consts.tile([H, DIMS], F32)
            b3_sb = consts.tile([R, DIMS], F32)
            for d in range(DIMS):
                nc.sync.dma_start(w0_sb[:, d * H:(d + 1) * H], w0[d, :, :])
                nc.sync.dma_start(w1_sb[:, d * H:(d + 1) * H], w1[d, :, :])
                nc.sync.dma_start(w2_sb[:, d * H:(d + 1) * H], w2[d, :, :])
                nc.sync.dma_start(w3_sb[:, d * R:(d + 1) * R], w3[d, :, :])
                nc.sync.dma_start(b0_sb[:, d:d + 1], b0[d, :, :])
                nc.sync.dma_start(b1_sb[:, d:d + 1], b1[d, :, :])
                nc.sync.dma_start(b2_sb[:, d:d + 1], b2[d, :, :])
                nc.sync.dma_start(b3_sb[:, d:d + 1], b3[d, :, :])

            import contextlib
            loop_cm = (tc.For_i(0, loop, 1,
                                hint_engines=(mybir.EngineType.PE,))
                       if loop > 1 else contextlib.nullcontext())
            with loop_cm:
                _emit_body(nc, tc, consts, work, stagep, mlp_ps, cp_ps,
                           x0, x1, x2, out,
                           w0_sb, w1_sb, w2_sb, w3_sb,
                           b0_sb, b1_sb, b2_sb, b3_sb, variant)

    nc.compile()
    return nc


def _emit_body(nc, tc, consts, work, stagep, mlp_ps, cp_ps,
               x0, x1, x2, out,
               w0_sb, w1_sb, w2_sb, w3_sb, b0_sb, b1_sb, b2_sb, b3_sb,
               variant="full"):
    if True:
        if True:
            f0_sb = consts.tile([R, NA], F32)
            f1_sb = consts.tile([R, N], F32)
            f2_sb = consts.tile([R, N], F32)

            def mlp(d, x_dram, npts, f_sb):
                # All intermediates are kept transposed: (features, points).
                xt = work.tile([1, npts], F32, name=f"xt_{d}", tag="xt")
                nc.sync.dma_start(xt[:], x_dram[:, :])
                ps = mlp_ps.tile([H, npts], F32, name=f"ps0_{d}", tag="mlp_ps")
                nc.tensor.matmul(ps[:], w0_sb[:, d * H:(d + 1) * H], xt[:],
                                 start=True, stop=True)
                h = work.tile([H, npts], F32, name=f"h0_{d}", tag="h")
                nc.scalar.activation(h[:], ps[:], TANH, bias=b0_sb[:, d:d + 1])
                for li, (w_sb, b_sb) in enumerate(((w1_sb, b1_sb), (w2_sb, b2_sb))):
                    ps = mlp_ps.tile([H, npts], F32, name=f"ps{li + 1}_{d}", tag="mlp_ps")
                    nc.tensor.matmul(ps[:], w_sb[:, d * H:(d + 1) * H], h[:],
                                     start=True, stop=True)
                    h = work.tile([H, npts], F32, name=f"h{li + 1}_{d}", tag="h")
                    nc.scalar.activation(h[:], ps[:], TANH, bias=b_sb[:, d:d + 1])
                ps = mlp_ps.tile([R, npts], F32, name=f"psf_{d}", tag="mlp_ps")
                nc.tensor.matmul(ps[:], w3_sb[:, d * R:(d + 1) * R], h[:],
                                 start=True, stop=True)
                nc.scalar.activation(f_sb[:], ps[:], IDENT, bias=b3_sb[:, d:d + 1])

            mlp(0, x0, NA, f0_sb)
            mlp(1, x1, N, f1_sb)
            mlp(2, x2, N, f2_sb)

            # Khatri-Rao: kr[r, a*N + b] = f0[r, a] * f1[r, b]
            kr_sb = consts.tile([R, NROWS], F32)
            for a in range(NA):
                nc.vector.tensor_scalar_mul(
                    kr_sb[:, a * N:(a + 1) * N], f1_sb[:], f0_sb[:, a:a + 1])

            # CP reconstruction: out rows in chunks of 128, batched DMAs out.
            outv = out[:, :].rearrange("(m p) c -> p m c", p=MCH)
            for g in range(NCHUNK // DMA_BATCH):
                stg = stagep.tile([MCH, DMA_BATCH * N], F32, name="stg", tag="stg")
                for k in range(DMA_BATCH):
                    m = g * DMA_BATCH + k
                    if variant != "dma_only":
                        ps = cp_ps.tile([MCH, N], F32, name="cps", tag="cps")
                        nc.tensor.matmul(ps[:], kr_sb[:, m * MCH:(m + 1) * MCH],
                                         f2_sb[:], start=True, stop=True)
                        if k % 2 == 0:
                            nc.vector.tensor_copy(stg[:, k * N:(k + 1) * N], ps[:])
                        else:
                            nc.scalar.copy(stg[:, k * N:(k + 1) * N], ps[:])
                if variant != "no_dma":
                    nc.sync.dma_start(
                        outv[:, g * DMA_BATCH:(g + 1) * DMA_BATCH, :],
                        stg.rearrange("p (m c) -> p m c", c=N),
                    )


def _get_program():
    global _PROG
    if _PROG is None:
        _PROG = _build_program()
    return _PROG


def _make_in_maps(xs, W0, b0, W1, b1, W2, b2, W3, b3):
    f = lambda x: np.ascontiguousarray(np.asarray(x), dtype=np.float32)
    xs = f(xs)
    base = {
        "x1": f(xs[1, :, 0].reshape(1, N)),
        "x2": f(xs[2, :, 0].reshape(1, N)),
        "w0": f(W0), "w1": f(W1), "w2": f(W2), "w3": f(W3),
        "b0": f(np.asarray(b0).reshape(DIMS, H, 1)),
        "b1": f(np.asarray(b1).reshape(DIMS, H, 1)),
        "b2": f(np.asarray(b2).reshape(DIMS, H, 1)),
        "b3": f(np.asarray(b3).reshape(DIMS, R, 1)),
    }
    in_maps = []
    for i in range(NCORES):
        m = dict(base)
        m["x0"] = f(xs[0, i * NA:(i + 1) * NA, 0].reshape(1, NA))
        in_maps.append(m)
    return in_maps


def run_spmd(inputs_kwargs, **run_kwargs):
    """Build (cached) program, run on all 8 cores; returns BassKernelResults."""
    nc = _get_program()
    in_maps = _make_in_maps(**inputs_kwargs)
    return run_bass_kernel_spmd(nc, in_maps, core_ids=list(range(NCORES)),
                                **run_kwargs)


def kernel(xs, W0, b0, W1, b1, W2, b2, W3, b3):
    res = run_spmd(dict(xs=xs, W0=W0, b0=b0, W1=W1, b1=b1,
                        W2=W2, b2=b2, W3=W3, b3=b3))
    slabs = [r["out"].reshape(NA, N, N) for r in res.results]
    return np.concatenate(slabs, axis=0)


# revision 6
# speedup vs baseline: 16.4768x; 1.4104x over previous
"""Trainium2 Bass kernel for the CP-PINN tensor reconstruction problem.

Computes, for xs (3,320,1) and three per-axis MLP weight stacks:
    f_d = MLP_d(xs[d])            (320, 64)   [tanh MLP: 1->128->128->128->64]
    out[a,b,c] = sum_r f_0[a,r] * f_1[b,r] * f_2[c,r]   ->  (320, 320, 320) f32

Strategy: data-parallel over the output's first axis across 8 NeuronCores
(40 a-points per core, no collectives). Each core:
  - computes the three MLPs in transposed (rank-major) layout on the
    TensorEngine + ScalarEngine (tanh), duplicating each factor matrix into
    both partition halves (rows 0-63 and 64-127) via column-group-tiled
    final-layer matmuls,
  - builds the Khatri-Rao product kr[r, a*320+b] = f0[r,a]*f1[r,b] with
    VectorEngine tensor_scalar ops, split across the two partition halves,
  - reconstructs its (40*320, 320) output slab with 100 K=64 matmuls
    kr_chunk(64,128)^T @ f2(64,320) issued as 50 concurrent pairs on PE row
    groups 0 and 64 (so LDWEIGHTS overlaps in-flight matmuls and the two
    row groups compute concurrently), evacuating PSUM via Vector/Scalar
    copies into SBUF staging tiles, and streams them to HBM with batched
    1.6 MB DMAs (the kernel is HBM-write-bound: 16.4 MB/core).
"""

import sys

if "/opt/trn_rl_repo" not in sys.path:
    sys.path.insert(0, "/opt/trn_rl_repo")

import numpy as np

import concourse.bacc as bacc
import concourse.mybir as mybir
from concourse import tile
from concourse.bass_utils import run_bass_kernel_spmd

DIMS = 3
N = 320          # points per coordinate axis
R = 64           # CP rank
H = 128          # hidden width
NCORES = 8
NA = N // NCORES          # a-points per core (40)
NROWS = NA * N            # output rows per core (12800)
MCH = 128                 # (a,b)-rows per matmul chunk
NCHUNK = NROWS // MCH     # 100
NPAIR = NCHUNK // 2       # 50 low/high chunk pairs
DMA_BATCH = 10            # chunks per output DMA (1.6 MB each)
NGRP = NPAIR // DMA_BATCH  # 5 groups; each emits one low + one high DMA

F32 = mybir.dt.float32
TANH = mybir.ActivationFunctionType.Tanh
IDENT = mybir.ActivationFunctionType.Identity

_PROG = None


def _build_program(loop=1, variant="full"):
    """loop>1 wraps the whole compute body in a Tile hardware For_i that
    repeats it `loop` times inside one NEFF launch — benchmarking only."""
    nc = bacc.Bacc("TRN2", target_bir_lowering=False)

    x0 = nc.dram_tensor("x0", [1, NA], F32, kind="ExternalInput")
    x1 = nc.dram_tensor("x1", [1, N], F32, kind="ExternalInput")
    x2 = nc.dram_tensor("x2", [1, N], F32, kind="ExternalInput")
    w0 = nc.dram_tensor("w0", [DIMS, 1, H], F32, kind="ExternalInput")
    w1 = nc.dram_tensor("w1", [DIMS, H, H], F32, kind="ExternalInput")
    w2 = nc.dram_tensor("w2", [DIMS, H, H], F32, kind="ExternalInput")
    w3 = nc.dram_tensor("w3", [DIMS, H, R], F32, kind="ExternalInput")
    b0 = nc.dram_tensor("b0", [DIMS, H, 1], F32, kind="ExternalInput")
    b1 = nc.dram_tensor("b1", [DIMS, H, 1], F32, kind="ExternalInput")
    b2 = nc.dram_tensor("b2", [DIMS, H, 1], F32, kind="ExternalInput")
    b3 = nc.dram_tensor("b3", [DIMS, R, 1], F32, kind="ExternalInput")
    out = nc.dram_tensor("out", [NROWS, N], F32, kind="ExternalOutput")

    with tile.TileContext(nc) as tc:
        with (
            tc.tile_pool(name="consts", bufs=1) as consts,
            tc.tile_pool(name="work", bufs=2) as work,
            tc.tile_pool(name="stage", bufs=3) as stagep,
            tc.tile_pool(name="mlp_ps", bufs=2, space="PSUM") as mlp_ps,
            tc.tile_pool(name="cp_ps", bufs=3, space="PSUM") as cp_ps,
        ):
            # Weights/biases for the 3 per-axis MLPs, packed along the free dim.
            w0_sb = consts.tile([1, DIMS * H], F32)
            w1_sb = consts.tile([H, DIMS * H], F32)
            w2_sb = consts.tile([H, DIMS * H], F32)
            w3_sb = consts.tile([H, DIMS * R], F32)
            b0_sb = consts.tile([H, DIMS], F32)
            b1_sb = consts.tile([H, DIMS], F32)
            b2_sb = consts.tile([H, DIMS], F32)
            b3_sb = consts.tile([2 * R, DIMS], F32)  # duplicated in both halves
            for d in range(DIMS):
                nc.sync.dma_start(w0_sb[:, d * H:(d + 1) * H], w0[d, :, :])
                nc.sync.dma_start(w1_sb[:, d * H:(d + 1) * H], w1[d, :, :])
                nc.sync.dma_start(w2_sb[:, d * H:(d + 1) * H], w2[d, :, :])
                nc.sync.dma_start(w3_sb[:, d * R:(d + 1) * R], w3[d, :, :])
                nc.sync.dma_start(b0_sb[:, d:d + 1], b0[d, :, :])
                nc.sync.dma_start(b1_sb[:, d:d + 1], b1[d, :, :])
                nc.sync.dma_start(b2_sb[:, d:d + 1], b2[d, :, :])
                nc.sync.dma_start(b3_sb[0:R, d:d + 1], b3[d, :, :])
                nc.sync.dma_start(b3_sb[R:2 * R, d:d + 1], b3[d, :, :])

            import contextlib
            loop_cm = (tc.For_i(0, loop, 1,
                                hint_engines=(mybir.EngineType.PE,))
                       if loop > 1 else contextlib.nullcontext())
            with loop_cm:
                _emit_body(nc, tc, consts, work, stagep, mlp_ps, cp_ps,
                           x0, x1, x2, out,
                           w0_sb, w1_sb, w2_sb, w3_sb,
                           b0_sb, b1_sb, b2_sb, b3_sb, variant)

    nc.compile()
    return nc


def _emit_body(nc, tc, consts, work, stagep, mlp_ps, cp_ps,
               x0, x1, x2, out,
               w0_sb, w1_sb, w2_sb, w3_sb, b0_sb, b1_sb, b2_sb, b3_sb,
               variant="full"):
    # Factor matrices in rank-major layout, duplicated across both
    # partition halves: f[0:64] == f[64:128].
    f0_sb = consts.tile([2 * R, NA], F32)
    f1_sb = consts.tile([2 * R, N], F32)
    f2_sb = consts.tile([2 * R, N], F32)

    def mlp(d, x_dram, npts, f_sb):
        # All intermediates are kept transposed: (features, points).
        xt = work.tile([1, npts], F32, name=f"xt_{d}", tag="xt")
        nc.sync.dma_start(xt[:], x_dram[:, :])
        ps = mlp_ps.tile([H, npts], F32, name=f"ps0_{d}", tag="mlp_ps")
        nc.tensor.matmul(ps[:], w0_sb[:, d * H:(d + 1) * H], xt[:],
                         start=True, stop=True)
        h = work.tile([H, npts], F32, name=f"h0_{d}", tag="h")
        nc.scalar.activation(h[:], ps[:], TANH, bias=b0_sb[:, d:d + 1])
        for li, (w_sb, b_sb) in enumerate(((w1_sb, b1_sb), (w2_sb, b2_sb))):
            ps = mlp_ps.tile([H, npts], F32, name=f"ps{li + 1}_{d}", tag="mlp_ps")
            nc.tensor.matmul(ps[:], w_sb[:, d * H:(d + 1) * H], h[:],
                             start=True, stop=True)
            h = work.tile([H, npts], F32, name=f"h{li + 1}_{d}", tag="h")
            nc.scalar.activation(h[:], ps[:], TANH, bias=b_sb[:, d:d + 1])
        # Final layer: write the (R, npts) result into BOTH partition halves
        # of one PSUM tile via column-group tiling, then one bias-add.
        ps = mlp_ps.tile([2 * R, npts], F32, name=f"psf_{d}", tag="mlp_ps")
        nc.tensor.matmul(ps[0:R, :], w3_sb[:, d * R:(d + 1) * R], h[:],
                         start=True, stop=True, tile_position=(0, 0))
        nc.tensor.matmul(ps[R:2 * R, :], w3_sb[:, d * R:(d + 1) * R], h[:],
                         start=True, stop=True, tile_position=(0, R))
        nc.scalar.activation(f_sb[:], ps[:], IDENT, bias=b3_sb[:, d:d + 1])

    mlp(0, x0, NA, f0_sb)
    mlp(1, x1, N, f1_sb)
    mlp(2, x2, N, f2_sb)

    # Khatri-Rao: kr[r, a*N + b] = f0[r, a] * f1[r, b].
    # Low partition half holds a in [0, NA/2); high half a in [NA/2, NA).
    kr_sb = consts.tile([2 * R, NROWS // 2], F32)
    for a in range(NA // 2):
        ah = a + NA // 2
        nc.vector.tensor_scalar_mul(
            kr_sb[0:R, a * N:(a + 1) * N], f1_sb[0:R, :], f0_sb[0:R, a:a + 1])
        nc.vector.tensor_scalar_mul(
            kr_sb[R:2 * R, a * N:(a + 1) * N], f1_sb[R:2 * R, :],
            f0_sb[R:2 * R, ah:ah + 1])
    if variant == "mlp_kr":
        return

    # CP reconstruction: 50 low/high chunk pairs on PE row groups 0 / 64.
    # Low chunks cover global rows [0, NROWS/2); high chunks the rest.
    outv = out[:, :].rearrange("(m p) c -> p m c", p=MCH)
    for g in range(NGRP):
        stg_lo = stagep.tile([MCH, DMA_BATCH * N], F32, name="stg_lo", tag="stg_lo")
        stg_hi = stagep.tile([MCH, DMA_BATCH * N], F32, name="stg_hi", tag="stg_hi")
        for k in range(DMA_BATCH):
            t = g * DMA_BATCH + k
            ps_lo = cp_ps.tile([MCH, N], F32, name="cps_lo", tag="cps_lo")
            nc.tensor.matmul(ps_lo[:], kr_sb[0:R, t * MCH:(t + 1) * MCH],
                             f2_sb[0:R, :], start=True, stop=True)
            ps_hi = cp_ps.tile([MCH, N], F32, name="cps_hi", tag="cps_hi")
            nc.tensor.matmul(ps_hi[:], kr_sb[R:2 * R, t * MCH:(t + 1) * MCH],
                             f2_sb[R:2 * R, :], start=True, stop=True)
            if variant != "no_copy":
                nc.vector.tensor_copy(stg_lo[:, k * N:(k + 1) * N], ps_lo[:])
                nc.scalar.copy(stg_hi[:, k * N:(k + 1) * N], ps_hi[:])
        if variant not in ("no_dma", "no_copy"):
            nc.sync.dma_start(
                outv[:, g * DMA_BATCH:(g + 1) * DMA_BATCH, :],
                stg_lo.rearrange("p (m c) -> p m c", c=N),
            )
            nc.sync.dma_start(
                outv[:, NPAIR + g * DMA_BATCH:NPAIR + (g + 1) * DMA_BATCH, :],
                stg_hi.rearrange("p (m c) -> p m c", c=N),
            )


def _get_program():
    global _PROG
    if _PROG is None:
        _PROG = _build_program()
    return _PROG


def _make_in_maps(xs, W0, b0, W1, b1, W2, b2, W3, b3):
    f = lambda x: np.ascontiguousarray(np.asarray(x), dtype=np.float32)
    xs = f(xs)
    base = {
        "x1": f(xs[1, :, 0].reshape(1, N)),
        "x2": f(xs[2, :, 0].reshape(1, N)),
        "w0": f(W0), "w1": f(W1), "w2": f(W2), "w3": f(W3),
        "b0": f(np.asarray(b0).reshape(DIMS, H, 1)),
        "b1": f(np.asarray(b1).reshape(DIMS, H, 1)),
        "b2": f(np.asarray(b2).reshape(DIMS, H, 1)),
        "b3": f(np.asarray(b3).reshape(DIMS, R, 1)),
    }
    in_maps = []
    for i in range(NCORES):
        m = dict(base)
        m["x0"] = f(xs[0, i * NA:(i + 1) * NA, 0].reshape(1, NA))
        in_maps.append(m)
    return in_maps


def run_spmd(inputs_kwargs, **run_kwargs):
    """Build (cached) program, run on all 8 cores; returns BassKernelResults."""
    nc = _get_program()
    in_maps = _make_in_maps(**inputs_kwargs)
    return run_bass_kernel_spmd(nc, in_maps, core_ids=list(range(NCORES)),
                                **run_kwargs)


def kernel(xs, W0, b0, W1, b1, W2, b2, W3, b3):
    res = run_spmd(dict(xs=xs, W0=W0, b0=b0, W1=W1, b1=b1,
                        W2=W2, b2=b2, W3=W3, b3=b3))
    slabs = [r["out"].reshape(NA, N, N) for r in res.results]
    return np.concatenate(slabs, axis=0)


# revision 25
# speedup vs baseline: 18.1821x; 1.1035x over previous
"""Trainium2 Bass kernel for the CP-PINN tensor reconstruction problem.

Computes, for xs (3,320,1) and three per-axis MLP weight stacks:
    f_d = MLP_d(xs[d])            (320, 64)   [tanh MLP: 1->128->128->128->64]
    out[a,b,c] = sum_r f_0[a,r] * f_1[b,r] * f_2[c,r]   ->  (320, 320, 320) f32

Strategy: data-parallel over the output's first axis across 8 NeuronCores
(40 a-points per core, no collectives). Each core:
  - loads ALL weights/biases with a single host-packed DMA (one 694 KB
    transfer instead of ~30 small serialized ones),
  - computes the three MLPs in transposed (rank-major) layout, interleaved
    layer-by-layer across dims on TensorEngine + ScalarEngine (tanh),
    duplicating each factor matrix into both partition halves (rows 0-63
    and 64-127) via column-group-tiled final-layer matmuls,
  - builds the Khatri-Rao product kr[r, a*320+b] = f0[r,a]*f1[r,b], low
    half on VectorE / high half on ScalarE,
  - reconstructs its (40*320, 320) output slab with 100 K=64 matmuls
    kr_chunk(64,128)^T @ f2(64,320) as 50 low/high pairs on PE row groups
    0 / 64, evacuating PSUM via VectorE (low) / ScalarE (high) copies into
    SBUF staging tiles, streamed to HBM with batched DMAs (first group
    small for an early ramp). The kernel is HBM-write-bound: 16.4 MB/core.
"""

import sys

if "/opt/trn_rl_repo" not in sys.path:
    sys.path.insert(0, "/opt/trn_rl_repo")

import numpy as np

import concourse.bacc as bacc
import concourse.mybir as mybir
from concourse import tile
from concourse.bass_utils import run_bass_kernel_spmd

DIMS = 3
N = 320          # points per coordinate axis
R = 64           # CP rank
H = 128          # hidden width
NCORES = 8
NA = N // NCORES          # a-points per core (40)
NROWS = NA * N            # output rows per core (12800)
MCH = 128                 # (a,b)-rows per matmul chunk
NCHUNK = NROWS // MCH     # 100
NPAIR = NCHUNK // 2       # 50 low/high chunk pairs
GROUPS = (2, 4, 8, 12, 12, 8, 2, 2)   # chunks per output DMA, per stream
assert sum(GROUPS) == NPAIR

# Packed-weights column layout (one (128, WCOLS) f32 tensor):
#   [0,384)    w1 (3 x 128 cols)        [384,768)  w2
#   [768,960)  w3 (3 x 64 cols)
#   [960,963) b0  [963,966) b1  [966,969) b2  [969,972) b3 (dup both halves)
#   [972,1356) w0 (row 0 only, 3 x 128 cols)
W1_OFF, W2_OFF, W3_OFF = 0, 384, 768
B0_OFF, B1_OFF, B2_OFF, B3_OFF = 960, 963, 966, 969
W0_OFF, WCOLS = 972, 1356
# Packed-x layout: (1, 680) = x0(40) | x1(320) | x2(320)
X0_OFF, X1_OFF, X2_OFF, XCOLS = 0, NA, NA + N, NA + 2 * N

F32 = mybir.dt.float32
F32R = mybir.dt.float32r
TANH = mybir.ActivationFunctionType.Tanh
IDENT = mybir.ActivationFunctionType.Identity

_PROG = None


def _build_program(loop=1, variant="full"):
    """loop>1 wraps the whole compute body in a Tile hardware For_i that
    repeats it `loop` times inside one NEFF launch — benchmarking only."""
    nc = bacc.Bacc("TRN2", target_bir_lowering=False)

    xp = nc.dram_tensor("xp", [1, XCOLS], F32, kind="ExternalInput")
    wp = nc.dram_tensor("wp", [H, WCOLS], F32, kind="ExternalInput")
    out = nc.dram_tensor("out", [NROWS, N], F32, kind="ExternalOutput")

    with tile.TileContext(nc) as tc:
        with (
            tc.tile_pool(name="consts", bufs=1) as consts,
            tc.tile_pool(name="work", bufs=2) as work,
            tc.tile_pool(name="stage", bufs=3) as stagep,
            tc.tile_pool(name="mlp_ps", bufs=2, space="PSUM") as mlp_ps,
            tc.tile_pool(name="cp_ps", bufs=3, space="PSUM") as cp_ps,
        ):
            wp_sb = consts.tile([H, WCOLS], F32)
            nc.sync.dma_start(wp_sb[:], wp[:, :])

            import contextlib
            loop_cm = (tc.For_i(0, loop, 1,
                                hint_engines=(mybir.EngineType.PE,))
                       if loop > 1 else contextlib.nullcontext())
            with loop_cm:
                _emit_body(nc, tc, consts, work, stagep, mlp_ps, cp_ps,
                           xp, out, wp_sb, variant)

    nc.compile()
    return nc


def _emit_body(nc, tc, consts, work, stagep, mlp_ps, cp_ps,
               xp, out, wp_sb, variant="full"):
    if variant == "dma_only":
        outv = out[:, :].rearrange("(m p) c -> p m c", p=MCH)
        t = 0
        for g, gsz in enumerate(GROUPS):
            stg_lo = stagep.tile([MCH, max(GROUPS) * N], F32, name="stg_lo",
                                 tag="stg_lo")
            stg_hi = stagep.tile([MCH, max(GROUPS) * N], F32, name="stg_hi",
                                 tag="stg_hi")
            nc.vector.memset(stg_lo[:, 0:1], 1.0)
            nc.vector.memset(stg_hi[:, 0:1], 1.0)
            nc.sync.dma_start(
                outv[:, t:t + gsz, :],
                stg_lo[:, 0:gsz * N].rearrange("p (m c) -> p m c", c=N))
            nc.sync.dma_start(
                outv[:, NPAIR + t:NPAIR + t + gsz, :],
                stg_hi[:, 0:gsz * N].rearrange("p (m c) -> p m c", c=N))
            t += gsz
        return
    # Factor matrices in rank-major layout, duplicated across both
    # partition halves: f[0:64] == f[64:128].
    f0_sb = consts.tile([2 * R, NA], F32)
    f1_sb = consts.tile([2 * R, N], F32)
    f2_sb = consts.tile([2 * R, N], F32)

    warm = work.tile([1, 1], F32, name="warm", tag="warm")
    nc.vector.memset(warm[:], 0.0)
    nc.scalar.activation(warm[:], warm[:], TANH)

    xp_sb = work.tile([1, XCOLS], F32, name="xp_sb", tag="xp_sb")
    nc.sync.dma_start(xp_sb[:], xp[:, :])

    # The three MLPs interleaved layer-by-layer so PE never waits on the
    # ScalarEngine tanh of the same dim (PE executes in program order).
    dims = [(0, X0_OFF, NA, f0_sb), (1, X1_OFF, N, f1_sb), (2, X2_OFF, N, f2_sb)]
    h_cur = {d: xp_sb[:, xoff:xoff + npts] for d, xoff, npts, _ in dims}
    w_l0 = wp_sb[0:1, :]
    for li, (w_off, b_off, w_ap, wid) in enumerate((
            (W0_OFF, B0_OFF, w_l0, H), (W1_OFF, B1_OFF, wp_sb, H),
            (W2_OFF, B2_OFF, wp_sb, H))):
        for d, _, npts, _ in dims:
            ps = mlp_ps.tile([H, npts], F32, name=f"ps{li}_{d}", tag="mlp_ps")
            nc.tensor.matmul(ps[:], w_ap[:, w_off + d * wid:w_off + (d + 1) * wid],
                             h_cur[d][:], start=True, stop=True)
            h = work.tile([H, npts], F32, name=f"h{li}_{d}", tag=f"h_{d}")
            nc.scalar.activation(h[:], ps[:], TANH,
                                 bias=wp_sb[:, b_off + d:b_off + d + 1])
            h_cur[d] = h
    # Final layer: write the (R, npts) result into BOTH partition halves
    # of one PSUM tile via column-group tiling, then one bias-add.
    for d, _, npts, f_sb in dims:
        w3 = wp_sb[:, W3_OFF + d * R:W3_OFF + (d + 1) * R]
        ps = mlp_ps.tile([2 * R, npts], F32, name=f"psf_{d}", tag="mlp_ps")
        nc.tensor.matmul(ps[0:R, :], w3, h_cur[d][:],
                         start=True, stop=True, tile_position=(0, 0))
        nc.tensor.matmul(ps[R:2 * R, :], w3, h_cur[d][:],
                         start=True, stop=True, tile_position=(0, R))
        nc.scalar.activation(f_sb[:], ps[:], IDENT,
                             bias=wp_sb[:, B3_OFF + d:B3_OFF + d + 1])

    if variant == "mlp_only":
        # consume f tiles so Tile releases are valid
        sink = work.tile([2 * R, N], F32, name="sink", tag="sink")
        nc.vector.tensor_copy(sink[:], f2_sb[:])
        nc.vector.tensor_copy(sink[:], f1_sb[:])
        nc.vector.tensor_copy(sink[:, 0:NA], f0_sb[:])
        return

    # Khatri-Rao: kr[r, a*N + b] = f0[r, a] * f1[r, b].
    # Low partition half holds a in [0, NA/2); high half a in [NA/2, NA).
    # Low half on VectorE, high half on ScalarE, in parallel. Ops are
    # emitted just-in-time per DMA group (engines are in-order; emitting
    # all KR first would delay the first copies by the whole KR phase).
    kr_sb = consts.tile([2 * R, NROWS // 2], F32)
    kr_emitted = 0

    def emit_kr_upto(a_need):
        nonlocal kr_emitted
        while kr_emitted < min(a_need, NA // 2):
            a = kr_emitted
            ah = a + NA // 2
            nc.vector.tensor_scalar_mul(
                kr_sb[0:R, a * N:(a + 1) * N], f1_sb[0:R, :],
                f0_sb[0:R, a:a + 1])
            nc.scalar.mul(
                kr_sb[R:2 * R, a * N:(a + 1) * N], f1_sb[R:2 * R, :],
                f0_sb[R:2 * R, ah:ah + 1])
            kr_emitted += 1

    if variant == "mlp_kr":
        emit_kr_upto(NA // 2)
        return

    # CP reconstruction: 50 low/high chunk pairs on PE row groups 0 / 64.
    # Low chunks cover global rows [0, NROWS/2); high chunks the rest.
    outv = out[:, :].rearrange("(m p) c -> p m c", p=MCH)
    t = 0
    for g, gsz in enumerate(GROUPS):
        # KR coverage for this group's chunk range plus one group lookahead
        nxt = GROUPS[g + 1] if g + 1 < len(GROUPS) else 0
        emit_kr_upto(-(-((t + gsz + nxt) * MCH) // N))
        stg_lo = stagep.tile([MCH, max(GROUPS) * N], F32, name="stg_lo",
                             tag="stg_lo")
        stg_hi = stagep.tile([MCH, max(GROUPS) * N], F32, name="stg_hi",
                             tag="stg_hi")
        t0 = t
        for k in range(gsz):
            ps_lo = cp_ps.tile([MCH, N], F32, name="cps_lo", tag="cps_lo")
            nc.tensor.matmul(ps_lo[:], kr_sb[0:R, t * MCH:(t + 1) * MCH],
                             f2_sb[0:R, :], start=True, stop=True)
            ps_hi = cp_ps.tile([MCH, N], F32, name="cps_hi", tag="cps_hi")
            nc.tensor.matmul(ps_hi[:], kr_sb[R:2 * R, t * MCH:(t + 1) * MCH],
                             f2_sb[R:2 * R, :], start=True, stop=True)
            if variant != "no_copy":
                eng_copy = (nc.vector.tensor_copy if t % 2 == 0
                            else nc.scalar.copy)
                eng_copy(stg_lo[:, k * N:(k + 1) * N], ps_lo[:])
                eng_copy(stg_hi[:, k * N:(k + 1) * N], ps_hi[:])
            t += 1
        if variant not in ("no_dma", "no_copy"):
            nc.sync.dma_start(
                outv[:, t0:t0 + gsz, :],
                stg_lo[:, 0:gsz * N].rearrange("p (m c) -> p m c", c=N),
            )
            nc.scalar.dma_start(
                outv[:, NPAIR + t0:NPAIR + t0 + gsz, :],
                stg_hi[:, 0:gsz * N].rearrange("p (m c) -> p m c", c=N),
            )


def _get_program():
    global _PROG
    if _PROG is None:
        _PROG = _build_program()
    return _PROG


def _pack_weights(W0, b0, W1, b1, W2, b2, W3, b3):
    wp = np.zeros((H, WCOLS), np.float32)
    for d in range(DIMS):
        wp[:, W1_OFF + d * H:W1_OFF + (d + 1) * H] = W1[d]
        wp[:, W2_OFF + d * H:W2_OFF + (d + 1) * H] = W2[d]
        wp[:, W3_OFF + d * R:W3_OFF + (d + 1) * R] = W3[d]
        wp[:, B0_OFF + d] = b0[d]
        wp[:, B1_OFF + d] = b1[d]
        wp[:, B2_OFF + d] = b2[d]
        wp[0:R, B3_OFF + d] = b3[d]
        wp[R:2 * R, B3_OFF + d] = b3[d]
        wp[0, W0_OFF + d * H:W0_OFF + (d + 1) * H] = W0[d, 0]
    return wp


def _make_in_maps(xs, W0, b0, W1, b1, W2, b2, W3, b3):
    f = lambda x: np.ascontiguousarray(np.asarray(x), dtype=np.float32)
    xs = f(xs)
    wp = _pack_weights(f(W0), f(b0), f(W1), f(b1), f(W2), f(b2), f(W3), f(b3))
    in_maps = []
    for i in range(NCORES):
        x = np.empty((1, XCOLS), np.float32)
        x[0, X0_OFF:X0_OFF + NA] = xs[0, i * NA:(i + 1) * NA, 0]
        x[0, X1_OFF:X1_OFF + N] = xs[1, :, 0]
        x[0, X2_OFF:X2_OFF + N] = xs[2, :, 0]
        in_maps.append({"xp": x, "wp": wp})
    return in_maps


def run_spmd(inputs_kwargs, **run_kwargs):
    """Build (cached) program, run on all 8 cores; returns BassKernelResults."""
    nc = _get_program()
    in_maps = _make_in_maps(**inputs_kwargs)
    return run_bass_kernel_spmd(nc, in_maps, core_ids=list(range(NCORES)),
                                **run_kwargs)


def kernel(xs, W0, b0, W1, b1, W2, b2, W3, b3):
    res = run_spmd(dict(xs=xs, W0=W0, b0=b0, W1=W1, b1=b1,
                        W2=W2, b2=b2, W3=W3, b3=b3))
    slabs = [r["out"].reshape(NA, N, N) for r in res.results]
    return np.concatenate(slabs, axis=0)


# revision 28
# speedup vs baseline: 19.4590x; 1.0702x over previous
"""Trainium2 Bass kernel for the CP-PINN tensor reconstruction problem.

Computes, for xs (3,320,1) and three per-axis MLP weight stacks:
    f_d = MLP_d(xs[d])            (320, 64)   [tanh MLP: 1->128->128->128->64]
    out[a,b,c] = sum_r f_0[a,r] * f_1[b,r] * f_2[c,r]   ->  (320, 320, 320) f32

Strategy: data-parallel over the output's first axis across 8 NeuronCores
(40 a-points per core, no collectives). Each core:
  - loads ALL weights/biases with a single host-packed DMA (one 694 KB
    transfer instead of ~30 small serialized ones),
  - computes the three MLPs in transposed (rank-major) layout, interleaved
    layer-by-layer across dims on TensorEngine + ScalarEngine (tanh),
    duplicating each factor matrix into both partition halves (rows 0-63
    and 64-127) via column-group-tiled final-layer matmuls,
  - builds the Khatri-Rao product kr[r, a*320+b] = f0[r,a]*f1[r,b], low
    half on VectorE / high half on ScalarE,
  - reconstructs its (40*320, 320) output slab with 100 K=64 matmuls
    kr_chunk(64,128)^T @ f2(64,320) as 50 low/high pairs on PE row groups
    0 / 64, evacuating PSUM via VectorE (low) / ScalarE (high) copies into
    SBUF staging tiles, streamed to HBM with batched DMAs (first group
    small for an early ramp). The kernel is HBM-write-bound: 16.4 MB/core.
"""

import sys

if "/opt/trn_rl_repo" not in sys.path:
    sys.path.insert(0, "/opt/trn_rl_repo")

import numpy as np

import concourse.bacc as bacc
import concourse.mybir as mybir
from concourse import tile
from concourse.bass_utils import run_bass_kernel_spmd

DIMS = 3
N = 320          # points per coordinate axis
R = 64           # CP rank
H = 128          # hidden width
NCORES = 8
NA = N // NCORES          # a-points per core (40)
NROWS = NA * N            # output rows per core (12800)
MCH = 128                 # (a,b)-rows per matmul chunk
NCHUNK = NROWS // MCH     # 100
NPAIR = NCHUNK // 2       # 50 low/high chunk pairs
GROUPS = (1, 2, 4, 8, 12, 12, 8, 2, 1)   # chunks per output DMA, per stream
assert sum(GROUPS) == NPAIR

# Packed-weights column layout (one (128, WCOLS) f32 tensor):
#   [0,384)    w1 (3 x 128 cols)        [384,768)  w2
#   [768,960)  w3 (3 x 64 cols)
#   [960,963) b0  [963,966) b1  [966,969) b2  [969,972) b3 (dup both halves)
#   [972,1356) w0 (row 0 only, 3 x 128 cols)
W1_OFF, W2_OFF, W3_OFF = 0, 384, 768
B0_OFF, B1_OFF, B2_OFF, B3_OFF = 960, 963, 966, 969
W0_OFF, WCOLS = 972, 1356
# Packed-x layout: (1, 680) = x0(40) | x1(320) | x2(320)
X0_OFF, X1_OFF, X2_OFF, XCOLS = 0, NA, NA + N, NA + 2 * N

F32 = mybir.dt.float32
F32R = mybir.dt.float32r
TANH = mybir.ActivationFunctionType.Tanh
IDENT = mybir.ActivationFunctionType.Identity

_PROG = None


def _build_program(loop=1, variant="full"):
    """loop>1 wraps the whole compute body in a Tile hardware For_i that
    repeats it `loop` times inside one NEFF launch — benchmarking only."""
    nc = bacc.Bacc("TRN2", target_bir_lowering=False)

    xp = nc.dram_tensor("xp", [1, XCOLS], F32, kind="ExternalInput")
    wp = nc.dram_tensor("wp", [H, WCOLS], F32, kind="ExternalInput")
    out = nc.dram_tensor("out", [NROWS, N], F32, kind="ExternalOutput")

    with tile.TileContext(nc) as tc:
        with (
            tc.tile_pool(name="consts", bufs=1) as consts,
            tc.tile_pool(name="work", bufs=2) as work,
            tc.tile_pool(name="stage", bufs=3) as stagep,
            tc.tile_pool(name="mlp_ps", bufs=2, space="PSUM") as mlp_ps,
            tc.tile_pool(name="cp_ps", bufs=3, space="PSUM") as cp_ps,
        ):
            wp_sb = consts.tile([H, WCOLS], F32)
            nc.sync.dma_start(wp_sb[:], wp[:, :])

            import contextlib
            loop_cm = (tc.For_i(0, loop, 1,
                                hint_engines=(mybir.EngineType.PE,))
                       if loop > 1 else contextlib.nullcontext())
            with loop_cm:
                _emit_body(nc, tc, consts, work, stagep, mlp_ps, cp_ps,
                           xp, out, wp_sb, variant)

    nc.compile()
    return nc


def _emit_body(nc, tc, consts, work, stagep, mlp_ps, cp_ps,
               xp, out, wp_sb, variant="full"):
    if variant == "dma_only":
        outv = out[:, :].rearrange("(m p) c -> p m c", p=MCH)
        t = 0
        for g, gsz in enumerate(GROUPS):
            stg_lo = stagep.tile([MCH, max(GROUPS) * N], F32, name="stg_lo",
                                 tag="stg_lo")
            stg_hi = stagep.tile([MCH, max(GROUPS) * N], F32, name="stg_hi",
                                 tag="stg_hi")
            nc.vector.memset(stg_lo[:, 0:1], 1.0)
            nc.vector.memset(stg_hi[:, 0:1], 1.0)
            nc.sync.dma_start(
                outv[:, t:t + gsz, :],
                stg_lo[:, 0:gsz * N].rearrange("p (m c) -> p m c", c=N))
            nc.sync.dma_start(
                outv[:, NPAIR + t:NPAIR + t + gsz, :],
                stg_hi[:, 0:gsz * N].rearrange("p (m c) -> p m c", c=N))
            t += gsz
        return
    # Factor matrices in rank-major layout, duplicated across both
    # partition halves: f[0:64] == f[64:128].
    f0_sb = consts.tile([2 * R, NA], F32)
    f1_sb = consts.tile([2 * R, N], F32)
    f2_sb = consts.tile([2 * R, N], F32)

    warm = work.tile([1, 1], F32, name="warm", tag="warm")
    nc.vector.memset(warm[:], 0.0)
    nc.scalar.activation(warm[:], warm[:], TANH)

    xp_sb = work.tile([1, XCOLS], F32, name="xp_sb", tag="xp_sb")
    nc.sync.dma_start(xp_sb[:], xp[:, :])

    # The three MLPs interleaved layer-by-layer so PE never waits on the
    # ScalarEngine tanh of the same dim (PE executes in program order).
    dims = [(0, X0_OFF, NA, f0_sb), (1, X1_OFF, N, f1_sb), (2, X2_OFF, N, f2_sb)]
    h_cur = {d: xp_sb[:, xoff:xoff + npts] for d, xoff, npts, _ in dims}
    w_l0 = wp_sb[0:1, :]
    for li, (w_off, b_off, w_ap, wid) in enumerate((
            (W0_OFF, B0_OFF, w_l0, H), (W1_OFF, B1_OFF, wp_sb, H),
            (W2_OFF, B2_OFF, wp_sb, H))):
        for d, _, npts, _ in dims:
            ps = mlp_ps.tile([H, npts], F32, name=f"ps{li}_{d}", tag="mlp_ps")
            nc.tensor.matmul(ps[:], w_ap[:, w_off + d * wid:w_off + (d + 1) * wid],
                             h_cur[d][:], start=True, stop=True)
            h = work.tile([H, npts], F32, name=f"h{li}_{d}", tag=f"h_{d}")
            nc.scalar.activation(h[:], ps[:], TANH,
                                 bias=wp_sb[:, b_off + d:b_off + d + 1])
            h_cur[d] = h
    # Final layer: write the (R, npts) result into BOTH partition halves
    # of one PSUM tile via column-group tiling, then one bias-add.
    for d, _, npts, f_sb in dims:
        w3 = wp_sb[:, W3_OFF + d * R:W3_OFF + (d + 1) * R]
        ps = mlp_ps.tile([2 * R, npts], F32, name=f"psf_{d}", tag="mlp_ps")
        nc.tensor.matmul(ps[0:R, :], w3, h_cur[d][:],
                         start=True, stop=True, tile_position=(0, 0))
        nc.tensor.matmul(ps[R:2 * R, :], w3, h_cur[d][:],
                         start=True, stop=True, tile_position=(0, R))
        nc.scalar.activation(f_sb[:], ps[:], IDENT,
                             bias=wp_sb[:, B3_OFF + d:B3_OFF + d + 1])

    if variant == "mlp_only":
        # consume f tiles so Tile releases are valid
        sink = work.tile([2 * R, N], F32, name="sink", tag="sink")
        nc.vector.tensor_copy(sink[:], f2_sb[:])
        nc.vector.tensor_copy(sink[:], f1_sb[:])
        nc.vector.tensor_copy(sink[:, 0:NA], f0_sb[:])
        return

    # Khatri-Rao: kr[r, a*N + b] = f0[r, a] * f1[r, b].
    # Low partition half holds a in [0, NA/2); high half a in [NA/2, NA).
    # Low half on VectorE, high half on ScalarE, in parallel. Ops are
    # emitted just-in-time per DMA group (engines are in-order; emitting
    # all KR first would delay the first copies by the whole KR phase).
    kr_sb = consts.tile([2 * R, NROWS // 2], F32)
    kr_emitted = 0

    def emit_kr_upto(a_need):
        nonlocal kr_emitted
        while kr_emitted < min(a_need, NA // 2):
            a = kr_emitted
            ah = a + NA // 2
            nc.vector.tensor_scalar_mul(
                kr_sb[0:R, a * N:(a + 1) * N], f1_sb[0:R, :],
                f0_sb[0:R, a:a + 1])
            nc.scalar.mul(
                kr_sb[R:2 * R, a * N:(a + 1) * N], f1_sb[R:2 * R, :],
                f0_sb[R:2 * R, ah:ah + 1])
            kr_emitted += 1

    if variant == "mlp_kr":
        emit_kr_upto(NA // 2)
        return

    # CP reconstruction: 50 low/high chunk pairs on PE row groups 0 / 64.
    # Low chunks cover global rows [0, NROWS/2); high chunks the rest.
    outv = out[:, :].rearrange("(m p) c -> p m c", p=MCH)
    t = 0
    for g, gsz in enumerate(GROUPS):
        # KR coverage for this group's chunk range plus one group lookahead
        nxt = GROUPS[g + 1] if g + 1 < len(GROUPS) else 0
        emit_kr_upto(-(-((t + gsz + nxt) * MCH) // N))
        stg_lo = stagep.tile([MCH, max(GROUPS) * N], F32, name="stg_lo",
                             tag="stg_lo")
        stg_hi = stagep.tile([MCH, max(GROUPS) * N], F32, name="stg_hi",
                             tag="stg_hi")
        t0 = t
        for k in range(gsz):
            ps_lo = cp_ps.tile([MCH, N], F32, name="cps_lo", tag="cps_lo")
            nc.tensor.matmul(ps_lo[:], kr_sb[0:R, t * MCH:(t + 1) * MCH],
                             f2_sb[0:R, :], start=True, stop=True)
            ps_hi = cp_ps.tile([MCH, N], F32, name="cps_hi", tag="cps_hi")
            nc.tensor.matmul(ps_hi[:], kr_sb[R:2 * R, t * MCH:(t + 1) * MCH],
                             f2_sb[R:2 * R, :], start=True, stop=True)
            if variant != "no_copy":
                # ~60/40 pair split toward DVE (ACT also carries KR-hi + MLP)
                eng_copy = (nc.vector.tensor_copy if t % 5 < 3
                            else nc.scalar.copy)
                eng_copy(stg_lo[:, k * N:(k + 1) * N], ps_lo[:])
                eng_copy(stg_hi[:, k * N:(k + 1) * N], ps_hi[:])
            t += 1
        if variant not in ("no_dma", "no_copy"):
            nc.sync.dma_start(
                outv[:, t0:t0 + gsz, :],
                stg_lo[:, 0:gsz * N].rearrange("p (m c) -> p m c", c=N),
            )
            nc.scalar.dma_start(
                outv[:, NPAIR + t0:NPAIR + t0 + gsz, :],
                stg_hi[:, 0:gsz * N].rearrange("p (m c) -> p m c", c=N),
            )


def _get_program():
    global _PROG
    if _PROG is None:
        _PROG = _build_program()
    return _PROG


def _pack_weights(W0, b0, W1, b1, W2, b2, W3, b3):
    wp = np.zeros((H, WCOLS), np.float32)
    for d in range(DIMS):
        wp[:, W1_OFF + d * H:W1_OFF + (d + 1) * H] = W1[d]
        wp[:, W2_OFF + d * H:W2_OFF + (d + 1) * H] = W2[d]
        wp[:, W3_OFF + d * R:W3_OFF + (d + 1) * R] = W3[d]
        wp[:, B0_OFF + d] = b0[d]
        wp[:, B1_OFF + d] = b1[d]
        wp[:, B2_OFF + d] = b2[d]
        wp[0:R, B3_OFF + d] = b3[d]
        wp[R:2 * R, B3_OFF + d] = b3[d]
        wp[0, W0_OFF + d * H:W0_OFF + (d + 1) * H] = W0[d, 0]
    return wp


def _make_in_maps(xs, W0, b0, W1, b1, W2, b2, W3, b3):
    f = lambda x: np.ascontiguousarray(np.asarray(x), dtype=np.float32)
    xs = f(xs)
    wp = _pack_weights(f(W0), f(b0), f(W1), f(b1), f(W2), f(b2), f(W3), f(b3))
    in_maps = []
    for i in range(NCORES):
        x = np.empty((1, XCOLS), np.float32)
        x[0, X0_OFF:X0_OFF + NA] = xs[0, i * NA:(i + 1) * NA, 0]
        x[0, X1_OFF:X1_OFF + N] = xs[1, :, 0]
        x[0, X2_OFF:X2_OFF + N] = xs[2, :, 0]
        in_maps.append({"xp": x, "wp": wp})
    return in_maps


def run_spmd(inputs_kwargs, **run_kwargs):
    """Build (cached) program, run on all 8 cores; returns BassKernelResults."""
    nc = _get_program()
    in_maps = _make_in_maps(**inputs_kwargs)
    return run_bass_kernel_spmd(nc, in_maps, core_ids=list(range(NCORES)),
                                **run_kwargs)


def kernel(xs, W0, b0, W1, b1, W2, b2, W3, b3):
    res = run_spmd(dict(xs=xs, W0=W0, b0=b0, W1=W1, b1=b1,
                        W2=W2, b2=b2, W3=W3, b3=b3))
    slabs = [r["out"].reshape(NA, N, N) for r in res.results]
    return np.concatenate(slabs, axis=0)
